# revision 1
# baseline (speedup 1.0000x reference)
"""Trainium2 Bass kernel for nn_CustomConv_66769561583718.

Reference op (per batch element):
  out = conv(x, W, stride=2, dilation=2, VALID)            # 3x3 taps, 9 total
      + conv(x, bias1[o] * FUZ, stride=2, VALID)           # dense 5x5

Structure exploited:
  * Term 1 reads only even-even input pixels; it is expressed as 9
    shifted 1x1 convs (matmuls over the 128 input channels) accumulated
    in PSUM.
  * FUZ = 0.1 * (ones(5,5) - dilated3x3_ones), and the 5x5 kernel is
    constant over input channels, so term 2 is rank-1:
        term2[o, y, x] = bias1[o] * S[y, x]
        S = 0.1 * (box5(T) - dilated3(T)) at stride 2,
        T[h, w] = sum_c x[c, h, w]
    T comes from M=1 ones-matmuls; the separable box passes are DVE adds
    (w direction) and a small matmul (h direction); the rank-1 term is a
    K=1 matmul accumulated into the same PSUM group as term 1.

Sharding: data-parallel over batch, 2 batches per core on 8 cores.

Emission order is tuned so PE never waits long on the big input DMA:
x arrives in h-chunks and T-matmuls + conv tap groups are interleaved
to track chunk arrival. Bias matmuls (which need the fully-reduced S)
close each PSUM group late, once s_row is ready.
"""

import numpy as np

import concourse.bacc as bacc
import concourse.mybir as mybir
import concourse.tile as tile
from concourse.bass_utils import run_bass_kernel_spmd

import ml_dtypes

dt = mybir.dt

B, CIN, H, W = 16, 128, 112, 112
COUT = 256
N_CORES = 8
BPC = B // N_CORES          # batches per core
HO = WO = 54
HW = H * W                  # 12544
T_CHUNK = 448               # 4 h-rows; 28 chunks per batch
N_TCHUNK = HW // T_CHUNK    # 28
# h-row boundaries of the input DMA chunks (small first chunk so PE
# starts early; the rest big for DMA efficiency)
X_CHUNK_ROWS = [0, 4, 16, 32, 48, 64, 80, 96, 112]
N_XCHUNK = len(X_CHUNK_ROWS) - 1
YT = 9                      # y-rows per output tile
NTILE = HO // YT            # 6 tiles per (batch, half)
NSP = YT * WO               # 486 spatial positions per tile

# conv/T datapath dtype: dt.float32r (fp32-precision-ish, 1 col/cycle at
# N>=256) or dt.bfloat16 (halves input HBM traffic; host pre-casts)
CONV_DT = dt.bfloat16
# output dtype: dt.float32 (exact) or dt.bfloat16 (halves output HBM
# traffic; host upcasts after gather)
OUT_DT = dt.float32


def _np_conv_dt(conv_dt):
    return ml_dtypes.bfloat16 if conv_dt == dt.bfloat16 else np.float32


# ablation switch for benching: subset of {"t","s","c"}; "c" = conv taps,
# "t" = channel-sum matmuls, "s" = S chain + bias matmuls (needs "t")
_PARTS = "tsc"


def _build(conv_dt=CONV_DT, iters=1, parts=None):
    if parts is None:
        parts = _PARTS
    do_t = "t" in parts
    do_s = "s" in parts and do_t
    do_c = "c" in parts
    nc = bacc.Bacc(None, target_bir_lowering=False)

    x = nc.dram_tensor("x", [BPC, CIN, H, W], conv_dt, kind="ExternalInput")
    # wt[c, tap, o] = weight[o, c, ky, kx], tap = ky*3+kx
    wt = nc.dram_tensor("wt", [CIN, 9, COUT], conv_dt, kind="ExternalInput")
    bias = nc.dram_tensor("bias", [1, COUT], dt.bfloat16, kind="ExternalInput")
    # lmat[h, k*HO + y]: k=0 -> 0.1*[2y<=h<=2y+4], k=1 -> -0.1*[h-2y in 0,2,4]
    lmat = nc.dram_tensor("lmat", [H, 2 * HO], dt.float32, kind="ExternalInput")
    ones = nc.dram_tensor("ones", [CIN, 1], conv_dt, kind="ExternalInput")
    out = nc.dram_tensor("out", [BPC, COUT, HO, WO], OUT_DT, kind="ExternalOutput")

    with tile.TileContext(nc) as tc:
        with (
            tc.tile_pool(name="const", bufs=1) as cpool,
            tc.tile_pool(name="x", bufs=1) as xpool,
            tc.tile_pool(name="trow", bufs=1) as trowpool,
            tc.tile_pool(name="small", bufs=2) as spool,
            tc.tile_pool(name="outsb", bufs=4) as opool,
            tc.tile_pool(name="pts", bufs=2, space="PSUM") as pts,
            tc.tile_pool(name="psc", bufs=6, space="PSUM") as psc,
        ):
            x_sbs = [
                xpool.tile([CIN, HW], conv_dt, tag=f"x{b}", name=f"x_sb{b}")
                for b in range(BPC)
            ]

            def emit_x_chunks(b, cs):
                xc = x[b].rearrange("c h w -> c (h w)")
                for c in cs:
                    lo = X_CHUNK_ROWS[c] * W
                    hi = X_CHUNK_ROWS[c + 1] * W
                    nc.sync.dma_start(out=x_sbs[b][:, lo:hi], in_=xc[:, lo:hi])

            # batch0 chunk 0 first so PE starts ASAP, then consts + weights,
            # then the rest of batch0. Later batches prefetch inside the loop.
            ones_sb = cpool.tile([CIN, 1], conv_dt)
            nc.sync.dma_start(out=ones_sb[:], in_=ones[:])

            def emit_consts():
                bias_sb = cpool.tile([1, COUT], dt.bfloat16)
                nc.sync.dma_start(out=bias_sb[:], in_=bias[:])
                lmat_sb = cpool.tile([H, 2 * HO], dt.float32)
                nc.sync.dma_start(out=lmat_sb[:], in_=lmat[:])
                wt_sb = cpool.tile([CIN, 9 * COUT], conv_dt)
                nc.sync.dma_start(
                    out=wt_sb[:], in_=wt[:].rearrange("c t o -> c (t o)")
                )
                return bias_sb, lmat_sb, wt_sb

            def emit_body(bias_sb, lmat_sb, wt_sb):
                for b in range(BPC):
                    x_sb = x_sbs[b]
                    xv = x_sb[:].rearrange("c (h w) -> c h w", w=W)

                    t_row = trowpool.tile([1, HW], dt.float32, tag="t_row",
                                          name="t_row")

                    def emit_t_chunks(ks):
                        for k in ks:
                            pt = pts.tile([1, T_CHUNK], dt.float32, tag="ts",
                                          name="pt")
                            nc.tensor.matmul(
                                out=pt[:],
                                lhsT=ones_sb[:],
                                rhs=x_sb[:, k * T_CHUNK : (k + 1) * T_CHUNK],
                                start=True,
                                stop=True,
                            )
                            sl = t_row[0:1, k * T_CHUNK : (k + 1) * T_CHUNK]
                            if k % 2 == 0:
                                nc.scalar.copy(out=sl, in_=pt[:])
                            else:
                                nc.vector.tensor_copy(out=sl, in_=pt[:])

                    def emit_taps(half, ti):
                        y0 = ti * YT
                        pc = psc.tile([128, NSP], dt.float32, tag="pc", name="pc")
                        for tap in range(9):
                            ky, kx = divmod(tap, 3)
                            h0 = 2 * y0 + 2 * ky
                            nc.tensor.matmul(
                                out=pc[:],
                                lhsT=wt_sb[
                                    :,
                                    tap * COUT + half * 128 :
                                    tap * COUT + half * 128 + 128,
                                ],
                                rhs=xv[
                                    :, h0 : h0 + 17 : 2, 2 * kx : 2 * kx + 107 : 2
                                ],
                                start=(tap == 0),
                                stop=(tap == 8 and not do_s),
                            )
                        return pc

                    def emit_bias_and_evict(half, ti, pc, s_row):
                        y0 = ti * YT
                        if do_s:
                            nc.tensor.matmul(
                                out=pc[:],
                                lhsT=bias_sb[0:1, half * 128 : half * 128 + 128],
                                rhs=s_row[0:1, y0 * WO : (y0 + YT) * WO],
                                start=False,
                                stop=True,
                            )
                        o_sb = opool.tile([128, NSP], OUT_DT, name="o_sb")
                        nc.vector.tensor_copy(out=o_sb[:], in_=pc[:])
                        nc.sync.dma_start(
                            out=out[
                                b, half * 128 : half * 128 + 128, y0 : y0 + YT, :
                            ],
                            in_=o_sb[:],
                        )

                    t_hw = spool.tile([H, W], dt.float32, tag="t_hw", name="t_hw")
                    c53 = spool.tile([H, 2 * HO], dt.float32, tag="c53", name="c53")
                    tmp = spool.tile([H, WO], dt.float32, tag="ctmp", name="tmp")

                    def emit_t_half(r0, r1):
                        # reshape rows [r0:r1] of T and run the w-direction
                        # box passes on them (C5 | C3 into c53)
                        nc.sync.dma_start(
                            out=t_hw[r0:r1, :], in_=t_row[0:1, r0 * W : r1 * W]
                        )
                        t = t_hw
                        nc.vector.tensor_add(
                            out=tmp[r0:r1, :],
                            in0=t[r0:r1, 0:107:2],
                            in1=t[r0:r1, 2:109:2],
                        )
                        nc.vector.tensor_add(
                            out=c53[r0:r1, HO : 2 * HO],
                            in0=tmp[r0:r1, :],
                            in1=t[r0:r1, 4:111:2],
                        )
                        nc.vector.tensor_add(
                            out=tmp[r0:r1, :],
                            in0=t[r0:r1, 1:108:2],
                            in1=t[r0:r1, 3:110:2],
                        )
                        nc.vector.tensor_add(
                            out=c53[r0:r1, 0:HO],
                            in0=c53[r0:r1, HO : 2 * HO],
                            in1=tmp[r0:r1, :],
                        )

                    def emit_s_chain():
                        ps_s = pts.tile([HO, WO], dt.float32, tag="ts", name="ps_s")
                        nc.tensor.matmul(
                            out=ps_s[:],
                            lhsT=lmat_sb[:, 0:HO],
                            rhs=c53[:, 0:HO],
                            start=True,
                            stop=False,
                        )
                        nc.tensor.matmul(
                            out=ps_s[:],
                            lhsT=lmat_sb[:, HO : 2 * HO],
                            rhs=c53[:, HO : 2 * HO],
                            start=False,
                            stop=True,
                        )
                        s54 = spool.tile([HO, WO], dt.bfloat16, tag="s54",
                                         name="s54")
                        nc.vector.tensor_copy(out=s54[:], in_=ps_s[:])
                        s_row = spool.tile([1, HO * WO], dt.bfloat16, tag="s_row",
                                           name="s_row")
                        nc.sync.dma_start(out=s_row[:], in_=s54[:])
                        return s_row

                    # Emission order keeps <=6 PSUM conv groups open and
                    # tracks x chunk arrival.
                    open_groups = []
                    if do_t:
                        emit_t_chunks(range(0, 8))
                    if do_c:
                        pc = emit_taps(0, 0); open_groups.append((0, 0, pc))
                        pc = emit_taps(1, 0); open_groups.append((1, 0, pc))
                    if do_t:
                        emit_t_chunks(range(8, 16))
                    if do_s:
                        emit_t_half(0, 64)
                    if do_c:
                        pc = emit_taps(0, 1); open_groups.append((0, 1, pc))
                        pc = emit_taps(1, 1); open_groups.append((1, 1, pc))
                    if do_t:
                        emit_t_chunks(range(16, 28))
                    s_row = None
                    if do_s:
                        emit_t_half(64, 112)
                        s_row = emit_s_chain()
                    if do_c:
                        pc = emit_taps(0, 2); open_groups.append((0, 2, pc))
                        pc = emit_taps(1, 2); open_groups.append((1, 2, pc))
                    for h2, t2, pc2 in open_groups:
                        emit_bias_and_evict(h2, t2, pc2, s_row)
                    open_groups = []
                    if b + 1 < BPC:
                        emit_x_chunks(b + 1, range(N_XCHUNK))
                    if do_c:
                        for ti in range(3, NTILE):
                            for half in range(2):
                                pc = emit_taps(half, ti)
                                emit_bias_and_evict(half, ti, pc, s_row)

            if iters == 1:
                emit_x_chunks(0, [0])
                bias_sb, lmat_sb, wt_sb = emit_consts()
                emit_x_chunks(0, range(1, N_XCHUNK))
                emit_body(bias_sb, lmat_sb, wt_sb)
            else:
                bias_sb, lmat_sb, wt_sb = emit_consts()
                with tc.For_i(0, iters, 1):
                    emit_x_chunks(0, range(N_XCHUNK))
                    emit_body(bias_sb, lmat_sb, wt_sb)
    nc.finalize()
    return nc


_NC_CACHE = {}


def _get_nc(conv_dt=CONV_DT, iters=1, parts=None):
    key = (str(conv_dt), iters, parts or _PARTS)
    if key not in _NC_CACHE:
        _NC_CACHE[key] = _build(conv_dt, iters, parts)
    return _NC_CACHE[key]


def _host_inputs(input_, weight, bias1, conv_dt=CONV_DT):
    """Build per-core input maps (numpy only)."""
    np_dt = _np_conv_dt(conv_dt)
    input_ = np.asarray(input_, dtype=np.float32).astype(np_dt)
    weight = np.asarray(weight, dtype=np.float32)
    bias1 = np.asarray(bias1, dtype=np.float32)

    wt = np.ascontiguousarray(
        weight.transpose(1, 2, 3, 0).reshape(CIN, 9, COUT)
    ).astype(np_dt)  # [c, (ky kx), o]
    bias_b = bias1.reshape(1, COUT).astype(ml_dtypes.bfloat16)
    lmat = np.zeros((H, 2 * HO), np.float32)
    for y in range(HO):
        for d in range(5):
            lmat[2 * y + d, y] = 0.1
        for d in (0, 2, 4):
            lmat[2 * y + d, HO + y] = -0.1
    ones = np.ones((CIN, 1), np_dt)

    in_maps = []
    for core in range(N_CORES):
        xs = np.ascontiguousarray(input_[core * BPC : (core + 1) * BPC])
        in_maps.append(
            {"x": xs, "wt": wt, "bias": bias_b, "lmat": lmat, "ones": ones}
        )
    return in_maps


def kernel(input_, weight, bias1):
    nc = _get_nc()
    in_maps = _host_inputs(input_, weight, bias1)
    res = run_bass_kernel_spmd(nc, in_maps, core_ids=list(range(N_CORES)))
    out = np.concatenate([r["out"] for r in res.results], axis=0)
    return np.asarray(out, dtype=np.float32)



# revision 22
# speedup vs baseline: 1.7002x; 1.7002x over previous
"""Trainium2 Bass kernel for nn_CustomConv_66769561583718.

Reference op (per batch element):
  out = conv(x, W, stride=2, dilation=2, VALID)            # 3x3 taps, 9 total
      + conv(x, bias1[o] * FUZ, stride=2, VALID)           # dense 5x5

Structure exploited:
  * Term 1 reads only even-even input pixels x_e = x[:, ::2, ::2]; it is
    a plain 3x3 stride-1 conv on the 56x56 subgrid. TAPS_MODE:
      - "direct": 9 shifted 1x1 convs (matmuls) accumulated in PSUM.
      - "wino":   Winograd F(2,3) along w: 12 matmuls (4 xi * 3 ky) of
        27 output pairs per row, 2/3 the PE rows of direct. Input
        transform u (4 DVE/Pool ops per batch), output transform
        (A^T) fused with the bias term during PSUM eviction.
  * FUZ = 0.1 * (ones(5,5) - dilated3x3_ones), and the 5x5 kernel is
    constant over input channels, so term 2 is rank-1:
        term2[o, y, x] = bias1[o] * S[y, x],
        S = 0.1 * (box5(T) - dilated3(T)) at stride 2,
        T[h, w] = sum_c x[c, h, w]
    The channel sums are ones-matmuls split into even (E) and odd (O)
    w columns so the separable box passes read them directly; each
    matmul broadcasts into a 64-partition half of one PSUM bank so a
    single copy evicts a chunk's E+O. The rank-1 bias term is fused
    into PSUM eviction with scalar_tensor_tensor:
    out = S128*bias[o] + acc, where S128 is S broadcast across
    partitions by a gpsimd partition_broadcast.

Engine notes: gpsimd (Pool) cannot touch PSUM on HW, so PSUM evictions
run on Act/DVE and Pool handles SBUF-only transform arithmetic.

Sharding: data-parallel over batch, 2 batches per core on 8 cores.
Output is bf16 on device (halves output HBM traffic); host upcasts.
"""

import numpy as np

import concourse.bacc as bacc
import concourse.mybir as mybir
import concourse.tile as tile
from concourse.bass_utils import run_bass_kernel_spmd

import ml_dtypes

dt = mybir.dt
Alu = mybir.AluOpType

B, CIN, H, W = 16, 128, 112, 112
COUT = 256
N_CORES = 8
BPC = B // N_CORES          # batches per core
HO = WO = 54
HW = H * W                  # 12544
WE = W // 2                 # 56 even (or odd) columns per row
EO_ROWS = 8                 # h-rows per E/O psum chunk (8*56=448 fp32 <= bank)
N_EOCHUNK = H // EO_ROWS    # 14
# h-row boundaries of the input DMA chunks (small first chunk so PE
# starts early; the rest big for DMA efficiency)
X_CHUNK_ROWS = [0, 4, 16, 32, 48, 64, 80, 96, 112]
N_XCHUNK = len(X_CHUNK_ROWS) - 1
# direct mode tiling
YT = 9                      # y-rows per output tile
NTILE = HO // YT            # 6 tiles per (batch, half)
NSP = YT * WO               # 486 spatial positions per tile
# wino mode tiling
NT = 27                     # output pairs per row
YB = 18                     # y-rows per wino block (18*27=486 fp32 <= bank)
NBLK = HO // YB             # 3 blocks per (batch, half)
NSW = YB * NT               # 486

CONV_DT = dt.bfloat16       # conv datapath dtype (x, weights, u)
OUT_DT = dt.bfloat16        # device output dtype; host upcasts
TAPS_MODE = "direct"        # "direct" | "wino"


def _np_dt(bass_dt):
    return mybir.dt.np(bass_dt)


# ablation switch for benching: subset of {"t","s","c"}; "c" = conv taps,
# "t" = channel-sum matmuls, "s" = S chain + bias fusion (needs "t")
_PARTS = "tsc"


def _build(conv_dt=CONV_DT, iters=1, parts=None, taps_mode=None):
    if parts is None:
        parts = _PARTS
    if taps_mode is None:
        taps_mode = TAPS_MODE
    do_t = "t" in parts
    do_s = "s" in parts and do_t
    do_c = "c" in parts
    nc = bacc.Bacc(None, target_bir_lowering=False)

    x = nc.dram_tensor("x", [BPC, CIN, H, W], conv_dt, kind="ExternalInput")
    if taps_mode == "direct":
        # wt[c, tap, o] = weight[o, c, ky, kx], tap = ky*3+kx
        wt = nc.dram_tensor("wt", [CIN, 9, COUT], conv_dt,
                            kind="ExternalInput")
    else:
        # wtw[c, ky*4+xi, o] = G-transformed weights along kx
        wt = nc.dram_tensor("wt", [CIN, 12, COUT], conv_dt,
                            kind="ExternalInput")
    # biascol[o, half] = bias1[half*128 + o]; cols 2,3 = negated halves
    biascol = nc.dram_tensor("biascol", [128, 4], dt.float32,
                             kind="ExternalInput")
    # lmat[h, k*HO + y]: k=0 -> 0.1*[2y<=h<=2y+4], k=1 -> -0.1*[h-2y in 0,2,4]
    lmat = nc.dram_tensor("lmat", [H, 2 * HO], dt.float32, kind="ExternalInput")
    ones = nc.dram_tensor("ones", [CIN, 64], conv_dt, kind="ExternalInput")
    out = nc.dram_tensor("out", [BPC, COUT, HO, WO], OUT_DT,
                         kind="ExternalOutput")

    n_wt = 9 if taps_mode == "direct" else 12

    with tile.TileContext(nc) as tc:
        with (
            tc.tile_pool(name="const", bufs=1) as cpool,
            tc.tile_pool(name="x", bufs=1) as xpool,
            tc.tile_pool(name="rows", bufs=1) as rowpool,
            tc.tile_pool(name="small", bufs=2) as spool,
            tc.tile_pool(name="s128", bufs=2) as bpool,
            tc.tile_pool(name="tmps", bufs=2) as tpool,
            tc.tile_pool(name="outsb", bufs=4) as opool,
            tc.tile_pool(name="pts", bufs=2, space="PSUM") as pts,
            tc.tile_pool(name="psc", bufs=6, space="PSUM") as psc,
        ):
            x_sbs = [
                xpool.tile([CIN, HW], conv_dt, tag=f"x{b}", name=f"x_sb{b}")
                for b in range(BPC)
            ]
            if taps_mode == "wino":
                # u[c, (r*4 + xi)*27 + t] bf16
                ut_sbs = [
                    xpool.tile([CIN, WE * 4 * NT], conv_dt, tag=f"ut{b}",
                               name=f"ut_sb{b}")
                    for b in range(BPC)
                ]

            def emit_x_chunks(b, cs):
                xc = x[b].rearrange("c h w -> c (h w)")
                for c in cs:
                    lo = X_CHUNK_ROWS[c] * W
                    hi = X_CHUNK_ROWS[c + 1] * W
                    nc.sync.dma_start(out=x_sbs[b][:, lo:hi], in_=xc[:, lo:hi])

            ones_sb = cpool.tile([CIN, 64], conv_dt)
            nc.sync.dma_start(out=ones_sb[:], in_=ones[:])

            def emit_consts():
                biascol_sb = cpool.tile([128, 4], dt.float32)
                nc.sync.dma_start(out=biascol_sb[:], in_=biascol[:])
                lmat_sb = cpool.tile([H, 2 * HO], dt.float32)
                nc.sync.dma_start(out=lmat_sb[:], in_=lmat[:])
                wt_sb = cpool.tile([CIN, n_wt * COUT], conv_dt)
                nc.sync.dma_start(
                    out=wt_sb[:], in_=wt[:].rearrange("c t o -> c (t o)")
                )
                return biascol_sb, lmat_sb, wt_sb

            def emit_body(biascol_sb, lmat_sb, wt_sb):
                for b in range(BPC):
                    x_sb = x_sbs[b]
                    xv = x_sb[:].rearrange("c (h w) -> c h w", w=W)
                    # E broadcast into rows 0..63, O into rows 64..127;
                    # rows 0 and 64 are consumed by the reshape DMAs.
                    eo_row = rowpool.tile([128, H * WE], dt.bfloat16,
                                          tag="eo_row", name="eo_row")

                    def emit_eo_chunks(ks):
                        # E (even w) and O (odd w) channel sums: bf16
                        # ones-matmuls over strided x columns. Each matmul
                        # broadcasts its row into a 64-partition half (same
                        # PE cost: cost scales with the free dim), so one
                        # [128, 448] bank holds a chunk's E+O and a single
                        # copy evicts both.
                        for k in ks:
                            h0 = k * EO_ROWS
                            h1 = h0 + EO_ROWS
                            pt = pts.tile([128, EO_ROWS * WE], dt.float32,
                                          tag="ts", name="pt")
                            for par in range(2):
                                nc.tensor.matmul(
                                    out=pt[64 * par : 64 * par + 64, :],
                                    lhsT=ones_sb[:],
                                    rhs=xv[:, h0:h1, par : par + 111 : 2],
                                    start=True,
                                    stop=True,
                                )
                            r0 = k * EO_ROWS * WE
                            r1 = r0 + EO_ROWS * WE
                            sl = eo_row[:, r0:r1]
                            if k % 3 == 2:
                                nc.vector.tensor_copy(out=sl, in_=pt[:])
                            else:
                                nc.scalar.copy(out=sl, in_=pt[:])

                    eo_hw = spool.tile([H, 2 * WE], dt.bfloat16, tag="eo_hw",
                                       name="eo_hw")
                    c53 = spool.tile([H, 2 * HO], dt.float32, tag="c53",
                                     name="c53")
                    tmp = spool.tile([H, WO], dt.float32, tag="ctmp",
                                     name="tmp")

                    def emit_wpass():
                        # reshape E/O rows into [h, we] grids, then the
                        # w-direction box passes (Pool, SBUF-only):
                        #   D = E[x]+E[x+1]+E[x+2]       (dil3_w)
                        #   B = D + O[x]+O[x+1]          (box5_w)
                        nc.sync.dma_start(out=eo_hw[:, 0:WE],
                                          in_=eo_row[0:1, :])
                        nc.sync.dma_start(out=eo_hw[:, WE : 2 * WE],
                                          in_=eo_row[64:65, :])
                        e_hw = eo_hw[:, 0:WE]
                        o_hw = eo_hw[:, WE : 2 * WE]
                        nc.gpsimd.tensor_add(
                            out=tmp[:, :],
                            in0=e_hw[:, 0:54],
                            in1=e_hw[:, 1:55],
                        )
                        nc.gpsimd.tensor_add(
                            out=c53[:, HO : 2 * HO],
                            in0=tmp[:, :],
                            in1=e_hw[:, 2:56],
                        )
                        nc.gpsimd.tensor_add(
                            out=tmp[:, :],
                            in0=o_hw[:, 0:54],
                            in1=o_hw[:, 1:55],
                        )
                        nc.gpsimd.tensor_add(
                            out=c53[:, 0:HO],
                            in0=c53[:, HO : 2 * HO],
                            in1=tmp[:, :],
                        )

                    def emit_s_chain():
                        ps_s = pts.tile([HO, WO], dt.float32, tag="ts",
                                        name="ps_s")
                        nc.tensor.matmul(
                            out=ps_s[:],
                            lhsT=lmat_sb[:, 0:HO],
                            rhs=c53[:, 0:HO],
                            start=True,
                            stop=False,
                        )
                        nc.tensor.matmul(
                            out=ps_s[:],
                            lhsT=lmat_sb[:, HO : 2 * HO],
                            rhs=c53[:, HO : 2 * HO],
                            start=False,
                            stop=True,
                        )
                        s54 = spool.tile([HO, WO], dt.bfloat16, tag="s54",
                                         name="s54")
                        nc.scalar.copy(out=s54[:], in_=ps_s[:])
                        s_row = spool.tile([1, HO * WO], dt.bfloat16,
                                           tag="s_row", name="s_row")
                        nc.sync.dma_start(out=s_row[:], in_=s54[:])
                        s128 = bpool.tile([128, HO * WO], dt.bfloat16,
                                          tag="s128", name="s128")
                        nc.gpsimd.partition_broadcast(s128[:], s_row[:])
                        return s128

                    # ---------------- direct taps ----------------
                    def emit_taps(half, ti):
                        y0 = ti * YT
                        pc = psc.tile([128, NSP], dt.float32, tag="pc",
                                      name="pc")
                        for tap in range(9):
                            ky, kx = divmod(tap, 3)
                            h0 = 2 * y0 + 2 * ky
                            nc.tensor.matmul(
                                out=pc[:],
                                lhsT=wt_sb[
                                    :,
                                    tap * COUT + half * 128 :
                                    tap * COUT + half * 128 + 128,
                                ],
                                rhs=xv[
                                    :, h0 : h0 + 17 : 2,
                                    2 * kx : 2 * kx + 107 : 2
                                ],
                                start=(tap == 0),
                                stop=(tap == 8),
                            )
                        return pc

                    def emit_bias_and_evict(half, ti, pc, s128):
                        y0 = ti * YT
                        o_sb = opool.tile([128, NSP], OUT_DT, name="o_sb")
                        if do_s:
                            # in1 is PSUM -> DVE (gpsimd can't read PSUM)
                            nc.vector.scalar_tensor_tensor(
                                out=o_sb[:],
                                in0=s128[:, y0 * WO : (y0 + YT) * WO],
                                scalar=biascol_sb[:, half : half + 1],
                                in1=pc[:],
                                op0=Alu.mult,
                                op1=Alu.add,
                            )
                        else:
                            nc.vector.tensor_copy(out=o_sb[:], in_=pc[:])
                        nc.sync.dma_start(
                            out=out[
                                b, half * 128 : half * 128 + 128,
                                y0 : y0 + YT, :
                            ],
                            in_=o_sb[:],
                        )

                    # ---------------- winograd taps ----------------
                    def emit_ut(r0, r1):
                        # input transform u for x_e rows [r0, r1):
                        #   u0 = d0-d2, u1 = d1+d2, u2 = d2-d1, u3 = d1-d3
                        # d_k = x[:, 2r, 4t + 2k]
                        utv = ut_sbs[b][:].rearrange(
                            "c (r xi t) -> c r xi t", xi=4, t=NT
                        )
                        hs = slice(2 * r0, 2 * r1, 2)
                        d0 = xv[:, hs, 0:108:4]
                        d1 = xv[:, hs, 2:110:4]
                        d2 = xv[:, hs, 4:112:4]
                        d3 = xv[:, hs, 6:111:4]
                        nc.gpsimd.tensor_sub(
                            out=utv[:, r0:r1, 0, :], in0=d0, in1=d2)
                        nc.gpsimd.tensor_add(
                            out=utv[:, r0:r1, 1, :], in0=d1, in1=d2)
                        nc.gpsimd.tensor_sub(
                            out=utv[:, r0:r1, 2, :], in0=d2, in1=d1)
                        nc.gpsimd.tensor_sub(
                            out=utv[:, r0:r1, 3, :], in0=d1, in1=d3)

                    def emit_wtaps(half, blk):
                        y0 = blk * YB
                        utv = ut_sbs[b][:].rearrange(
                            "c (r xi t) -> c r xi t", xi=4, t=NT
                        )
                        ms = []
                        for xi in range(4):
                            m = psc.tile([128, NSW], dt.float32, tag="pc",
                                         name=f"m{xi}")
                            for ky in range(3):
                                kk = ky * 4 + xi
                                nc.tensor.matmul(
                                    out=m[:],
                                    lhsT=wt_sb[
                                        :,
                                        kk * COUT + half * 128 :
                                        kk * COUT + half * 128 + 128,
                                    ],
                                    rhs=utv[:, y0 + ky : y0 + ky + YB, xi, :],
                                    start=(ky == 0),
                                    stop=(ky == 2),
                                )
                            ms.append(m)
                        return ms

                    def emit_wevict(half, blk, ms, s128):
                        y0 = blk * YB
                        m0, m1, m2, m3 = ms
                        c1 = tpool.tile([128, NSW], dt.float32, tag="c1",
                                        name="c1")
                        c2 = tpool.tile([128, NSW], dt.float32, tag="c2",
                                        name="c2")
                        v = tpool.tile([128, NSW], dt.float32, tag="v",
                                       name="v")
                        u = tpool.tile([128, NSW], dt.float32, tag="u",
                                       name="u")
                        v2 = tpool.tile([128, NSW], dt.float32, tag="v2",
                                        name="v2")
                        w = tpool.tile([128, NSW], dt.float32, tag="w",
                                       name="w")
                        o_sb = opool.tile([128, YB * WO], OUT_DT, name="o_sb")
                        ov = o_sb[:].rearrange("p (y w) -> p y w", w=WO)
                        vv = v[:].rearrange("p (y t) -> p y t", t=NT)
                        uv = u[:].rearrange("p (y t) -> p y t", t=NT)
                        # Two-level chain to keep in-order engines from
                        # head-of-line blocking: level 1 drains PSUM fast
                        # (Act copies m1/m2; DVE STTs fold bias*S into
                        # m0/m3), level 2 is SBUF-only on Pool.
                        #   o_even = (biasS_e + m0) + m1 + m2
                        #   o_odd  = (m1 - m2) - (m3 - biasS_o)
                        nc.scalar.copy(out=c1[:], in_=m1[:])
                        nc.scalar.copy(out=c2[:], in_=m2[:])
                        if do_s:
                            s3 = s128[:, y0 * WO : (y0 + YB) * WO].rearrange(
                                "p (y w) -> p y w", w=WO
                            )
                            nc.vector.scalar_tensor_tensor(
                                out=vv[:],
                                in0=s3[:, :, 0:WO:2],
                                scalar=biascol_sb[:, half : half + 1],
                                in1=m0[:].rearrange("p (y t) -> p y t", t=NT),
                                op0=Alu.mult,
                                op1=Alu.add,
                            )
                            nc.vector.scalar_tensor_tensor(
                                out=uv[:],
                                in0=s3[:, :, 1:WO:2],
                                scalar=biascol_sb[:, 2 + half : 3 + half],
                                in1=m3[:].rearrange("p (y t) -> p y t", t=NT),
                                op0=Alu.mult,
                                op1=Alu.add,
                            )
                        else:
                            nc.vector.tensor_copy(out=v[:], in_=m0[:])
                            nc.vector.tensor_copy(out=u[:], in_=m3[:])
                        nc.gpsimd.tensor_add(out=v2[:], in0=v[:], in1=c1[:])
                        nc.gpsimd.tensor_add(
                            out=ov[:, :, 0:WO:2],
                            in0=v2[:].rearrange("p (y t) -> p y t", t=NT),
                            in1=c2[:].rearrange("p (y t) -> p y t", t=NT),
                        )
                        nc.gpsimd.tensor_sub(out=w[:], in0=c1[:], in1=c2[:])
                        nc.gpsimd.tensor_sub(
                            out=ov[:, :, 1:WO:2],
                            in0=w[:].rearrange("p (y t) -> p y t", t=NT),
                            in1=uv[:],
                        )
                        nc.sync.dma_start(
                            out=out[
                                b, half * 128 : half * 128 + 128,
                                y0 : y0 + YB, :
                            ],
                            in_=o_sb[:],
                        )

                    # ---------------- per-batch schedule ----------------
                    if do_t:
                        emit_eo_chunks(range(0, 7))
                    if do_c and taps_mode == "wino":
                        emit_ut(0, 24)
                    if do_t:
                        emit_eo_chunks(range(7, 14))
                    if do_c and taps_mode == "wino":
                        emit_ut(24, 56)
                    s128 = None
                    if do_s:
                        emit_wpass()
                        s128 = emit_s_chain()

                    if do_c and taps_mode == "direct":
                        open_groups = []
                        for ti in range(2):
                            for half in range(2):
                                pc = emit_taps(half, ti)
                                open_groups.append((half, ti, pc))
                        for h2, t2_, pc2 in open_groups:
                            emit_bias_and_evict(h2, t2_, pc2, s128)
                        if b + 1 < BPC:
                            emit_x_chunks(b + 1, range(N_XCHUNK))
                        for ti in range(2, NTILE):
                            for half in range(2):
                                pc = emit_taps(half, ti)
                                emit_bias_and_evict(half, ti, pc, s128)
                    elif do_c:
                        first = True
                        for blk in range(NBLK):
                            for half in range(2):
                                ms = emit_wtaps(half, blk)
                                emit_wevict(half, blk, ms, s128)
                            if first and b + 1 < BPC:
                                emit_x_chunks(b + 1, range(N_XCHUNK))
                                first = False
                    else:
                        if b + 1 < BPC:
                            emit_x_chunks(b + 1, range(N_XCHUNK))

            if iters == 1:
                emit_x_chunks(0, [0])
                biascol_sb, lmat_sb, wt_sb = emit_consts()
                emit_x_chunks(0, range(1, N_XCHUNK))
                emit_body(biascol_sb, lmat_sb, wt_sb)
            else:
                biascol_sb, lmat_sb, wt_sb = emit_consts()
                with tc.For_i(0, iters, 1):
                    emit_x_chunks(0, range(N_XCHUNK))
                    emit_body(biascol_sb, lmat_sb, wt_sb)
    nc.finalize()
    return nc


_NC_CACHE = {}


def _get_nc(conv_dt=CONV_DT, iters=1, parts=None, taps_mode=None):
    key = (str(conv_dt), iters, parts or _PARTS, taps_mode or TAPS_MODE)
    if key not in _NC_CACHE:
        _NC_CACHE[key] = _build(conv_dt, iters, parts, taps_mode)
    return _NC_CACHE[key]


def _host_inputs(input_, weight, bias1, conv_dt=CONV_DT, taps_mode=None):
    """Build per-core input maps (numpy only)."""
    if taps_mode is None:
        taps_mode = TAPS_MODE
    np_dt = _np_dt(conv_dt)
    input_ = np.asarray(input_, dtype=np.float32)
    weight = np.asarray(weight, dtype=np.float32)
    bias1 = np.asarray(bias1, dtype=np.float32)

    x_b = input_.astype(np_dt)

    if taps_mode == "direct":
        wt = np.ascontiguousarray(
            weight.transpose(1, 2, 3, 0).reshape(CIN, 9, COUT)
        ).astype(np_dt)  # [c, (ky kx), o]
    else:
        w0, w1, w2 = weight[..., 0], weight[..., 1], weight[..., 2]
        wtw = np.stack(
            [w0, 0.5 * (w0 + w1 + w2), 0.5 * (w0 - w1 + w2), w2], axis=-1
        )  # [o, c, ky, xi]
        wt = np.ascontiguousarray(
            wtw.transpose(1, 2, 3, 0).reshape(CIN, 12, COUT)
        ).astype(np_dt)  # [c, (ky xi), o]
    bcol = bias1.reshape(2, 128).T  # [o, half]
    biascol = np.ascontiguousarray(
        np.concatenate([bcol, -bcol], axis=1)
    ).astype(np.float32)  # [o, (half, neg half)]
    lmat = np.zeros((H, 2 * HO), np.float32)
    for y in range(HO):
        for d in range(5):
            lmat[2 * y + d, y] = 0.1
        for d in (0, 2, 4):
            lmat[2 * y + d, HO + y] = -0.1
    ones = np.ones((CIN, 64), np_dt)

    in_maps = []
    for core in range(N_CORES):
        sl = slice(core * BPC, (core + 1) * BPC)
        in_maps.append(
            {
                "x": np.ascontiguousarray(x_b[sl]),
                "wt": wt,
                "biascol": biascol,
                "lmat": lmat,
                "ones": ones,
            }
        )
    return in_maps


def kernel(input_, weight, bias1):
    nc = _get_nc()
    in_maps = _host_inputs(input_, weight, bias1)
    res = run_bass_kernel_spmd(nc, in_maps, core_ids=list(range(N_CORES)))
    out = np.concatenate([r["out"] for r in res.results], axis=0)
    return np.asarray(out, dtype=np.float32)
